# revision 1
# baseline (speedup 1.0000x reference)
"""AWLoss2D Trainium2 kernel.

Math summary (per sample, H=W=32):
  Z = full-2D-conv (doubly-blocked Toeplitz) matrix of target patch X, [3969, 1024]
  v = Z^T Z + eps*I  (BTTB, diagonalized-modulo-support by the 64x64 DFT)
  w = v^{-1} Z^T d   (d = centered zero-pad of recon patch)
  loss = 0.5*||T2D .* w|| / ||w||, summed over 24 samples.

Device algorithm: pipelined CG (Ghysels-Vanroose, fixed 12 iters) on
v w = b, where the matvec
  v p = P^T IFFT2( |FFT2(X)|^2 .* FFT2(P p) ) + eps p
is computed with explicit 64x64 DFT-matrix matmuls on the tensor engine
(F is symmetric, so stage S(G) := G^T F applied twice gives FFT2 with no
transposes; real-input Hermitian symmetry folds the column-frequency axis
to 33 bins and the inverse stages produce only the 32x32 corner).
24 samples = 8 cores x 3 lanes (4th lane duplicates the 3rd).
fp32 PIPECG at 12 iters vs f64 reference: total rel err ~7e-5.
"""

import numpy as np

H = W = 32
N = 64  # FFT grid
LANES = 4  # per core: 3 real samples + 1 duplicate
N_CORES = 8
ITERS = 12
EPS = 1e-4
F32 = np.float32

_NC_CACHE = {}


# ---------------------------------------------------------------- host consts
def _t2d_flat():
    xarr = np.linspace(-10.0, 10.0, H)
    yarr = np.linspace(-10.0, 10.0, W)
    xx, yy = np.meshgrid(xarr, yarr, indexing="ij")
    dispx = (H % 2 - 1) / 2.0
    dispy = (W % 2 - 1) / 2.0
    dx = (xarr[-1] - xarr[0]) / (H - 1)
    dy = (yarr[-1] - yarr[0]) / (W - 1)
    sx = sy = 1.0
    t = -(1.0 / (2.0 * np.pi * sx * sy)) * np.exp(
        -((xx - dx * dispx) ** 2 / (2 * sx**2) + (yy - dy * dispy) ** 2 / (2 * sy**2))
    )
    t = t + np.max(np.abs(t))
    return (t / np.max(np.abs(t))).astype(F32)


KF = 33  # folded frequency count (Hermitian symmetry of real-input FFT)


def _dft_consts():
    k = np.arange(N)
    Fc = np.exp(-2j * np.pi * np.outer(k, k) / N)  # symmetric
    Fr = Fc.real
    Fi = Fc.imag
    Gr = Fc.real / N       # conj(F)/N, real part
    Gi = -Fc.imag / N      # conj(F)/N, imag part
    # S1 rhs: full [Fr | Fi] (all 64 row-freqs k)
    CF = np.concatenate([Fr, Fi], axis=1)                      # [64,128]
    # S2 rhs: column-freqs folded to k2 in [0,33)
    CFh = np.concatenate([Fr[:, :KF], Fi[:, :KF]], axis=1)     # [64,66]
    CF2h = np.concatenate([-Fi[:, :KF], Fr[:, :KF]], axis=1)
    # T1 rhs: inverse over row-freq k (full 64), spatial rows r in [0,32) only
    CG1h = np.concatenate([Gr[:, :32], Gi[:, :32]], axis=1)    # [64,64]
    CG2h = np.concatenate([-Gi[:, :32], Gr[:, :32]], axis=1)
    # T2 rhs: inverse over folded k2 with Hermitian weights, cols c in [0,32)
    w = np.ones((KF, 1))
    w[1:32] = 2.0  # k2 = 0 and 32 are self-paired
    CGf1 = np.zeros((64, 32))
    CGf2 = np.zeros((64, 32))
    CGf1[:KF] = w * Gr[:KF, :32]
    CGf2[:KF] = w * (-Gi[:KF, :32])
    dup = lambda a: np.concatenate([a, a], axis=0).astype(F32)
    return tuple(dup(a) for a in (CF, CFh, CF2h, CG1h, CG2h, CGf1, CGf2))


# ---------------------------------------------------------------- bass program
def build_nc():
    import concourse.bass as bass
    import concourse.mybir as mybir
    import concourse.tile as tile
    from concourse import bacc

    dt = mybir.dt.float32
    Alu = mybir.AluOpType

    nc = bacc.Bacc("TRN2", target_bir_lowering=False)

    tq = nc.dram_tensor("tq", [LANES, H, W], dt, kind="ExternalInput").ap()
    rq = nc.dram_tensor("rq", [LANES, H, W], dt, kind="ExternalInput").ap()
    out = nc.dram_tensor("loss", [1, LANES], dt, kind="ExternalOutput").ap()

    const_np = _dft_consts()
    const_names = ("cf", "cfh", "cf2h", "cg1h", "cg2h", "cgf1", "cgf2")
    # merge every constant into one [128, sum] block -> single DMA at start
    t2d128 = np.zeros((128, 32), dtype=F32)
    t2d128[0:32] = _t2d_flat()
    all_consts = np.concatenate(list(const_np) + [t2d128], axis=1)
    cwidths = [a.shape[1] for a in const_np] + [32]
    allc_d = nc.inline_tensor(all_consts, "allc").ap()

    def corner(ap32wide, blk=N, w=32):
        # [32, nblk, w] view of the first 32 partitions, first w cols per block
        nblk = ap32wide.shape[1] // blk
        return ap32wide[0:32].rearrange("p (l c) -> p l c", l=nblk)[:, :, 0:w]

    def corner_q(grid, q):
        # lanes {q, q+2} corners of a [64,256] grid: [32, 2, 32]
        return grid[0:32].rearrange("p (l c) -> p l c", l=4)[:, q::2, 0:32]

    with tile.TileContext(nc) as tc:
        with (
            tc.tile_pool(name="consts", bufs=1) as consts,
            tc.tile_pool(name="state", bufs=1) as state,
            tc.tile_pool(name="loop", bufs=2) as loop,
            tc.tile_pool(name="psA", bufs=2, space="PSUM") as psA,
            tc.tile_pool(name="psB", bufs=2, space="PSUM") as psB,
            tc.tile_pool(name="psC", bufs=1, space="PSUM") as psC,
            tc.tile_pool(name="psS", bufs=1, space="PSUM") as psS,
        ):
            # ---------------- constants into SBUF (one DMA)
            CALL = consts.tile([128, all_consts.shape[1]], dt)
            nc.sync.dma_start(CALL[:], allc_d)
            offs = np.cumsum([0] + cwidths)
            cslices = [CALL[:, int(offs[i]) : int(offs[i + 1])]
                       for i in range(len(cwidths))]
            CF, CFh, CF2h, CG1h, CG2h, CGf1, CGf2, TqF = cslices
            Tq = TqF[0:32, :]
            ones = consts.tile([64, 64], dt)
            nc.any.memset(ones[:], 1.0)

            # ---------------- persistent state
            XDg = state.tile([64, 512], dt)    # target grids (lanes 0-3) +
                                               # padded recon grids (lanes 4-7)
            Dall = state.tile([128, 2 * 2 * KF], dt)  # |FFT2 X|^2 dup, per pair
            xv = state.tile([64, 256], dt)
            rv_a = state.tile([64, 256], dt)
            rv_b = state.tile([64, 256], dt)
            wv_a = state.tile([64, 256], dt)
            wv_b = state.tile([64, 256], dt)
            pv = state.tile([64, 256], dt)
            sv = state.tile([64, 256], dt)
            zv = state.tile([64, 256], dt)
            gam_t = [
                state.tile([1, 4], dt, tag=f"gam{i}", name=f"gam{i}")
                for i in range(2)
            ]
            alp_t = [
                state.tile([1, 4], dt, tag=f"alp{i}", name=f"alp{i}")
                for i in range(2)
            ]
            ps_dot = psS.tile([1, 8], dt, tag="sdot")    # gamma | delta
            ps_bc = psS.tile([32, 8], dt, tag="sbc")     # beta | alpha bcast

            for t in (XDg, xv, rv_a, rv_b, wv_a, wv_b, pv, sv, zv):
                nc.any.memset(t[:], 0.0)

            # load inputs into grid corners
            nc.sync.dma_start(
                corner(XDg[:, 0:256]), tq.rearrange("l h w -> h l w")
            )
            nc.sync.dma_start(
                XDg[15:47, 256:512].rearrange("p (l c) -> p l c", l=4)[:, :, 15:47],
                rq.rearrange("l h w -> h l w"),
            )

            # ---------------- helpers
            def fft_stage1(src, ps, npairs=2):
                # S1: for pair p: out[:, 128p:128p+128] = src_pair^T @ [Fr|Fi]
                for p in range(npairs):
                    nc.tensor.matmul(
                        ps[:, 128 * p : 128 * p + 128],
                        lhsT=src[:, 128 * p : 128 * p + 128],
                        rhs=CF[0:64, :],
                        start=True,
                        stop=True,
                    )

            def fft_stage2(Hsb, ps, npairs=2):
                # S2: per pair p, per half q (lane 2p+q at partitions 64q):
                #   out = Hre^T [Fr|Fi] + Him^T [-Fi|Fr], col-freqs folded to KF
                for p in range(npairs):
                    for q in range(2):
                        sl = slice(64 * q, 64 * q + 64)
                        dst = ps[sl, 66 * p : 66 * p + 66]
                        nc.tensor.matmul(
                            dst,
                            lhsT=Hsb[sl, 128 * p : 128 * p + 64],
                            rhs=CFh[sl, :],
                            start=True,
                            stop=False,
                        )
                        nc.tensor.matmul(
                            dst,
                            lhsT=Hsb[sl, 128 * p + 64 : 128 * p + 128],
                            rhs=CF2h[sl, :],
                            start=False,
                            stop=True,
                        )

            def ifft_stage1(Wsb, ps):
                # T1: out[k2, r] for k2 in [0,KF), r in [0,32):
                #   (T1re|T1im) = Wre^T [Gr|Gi] + Wim^T [-Gi|Gr]
                for p in range(2):
                    for q in range(2):
                        sl = slice(64 * q, 64 * q + 64)
                        dst = ps[64 * q : 64 * q + KF, 64 * p : 64 * p + 64]
                        nc.tensor.matmul(
                            dst,
                            lhsT=Wsb[sl, 66 * p : 66 * p + KF],
                            rhs=CG1h[sl, :],
                            start=True,
                            stop=False,
                        )
                        nc.tensor.matmul(
                            dst,
                            lhsT=Wsb[sl, 66 * p + KF : 66 * p + 66],
                            rhs=CG2h[sl, :],
                            start=False,
                            stop=True,
                        )

            def ifft_stage2(Tsb, ps3_pair):
                # T2 (real part, 32x32 corner, Hermitian-weighted fold over
                # k2 in [0,KF)): lane l=2p+q -> ps3_pair[q][0:32, 32p:32p+32].
                # q=0/q=1 use different row groups; they must land in
                # different PSUM tiles (mixed tile_position into one PSUM
                # bank faults).  q-outer: lanes (0,2) finish first so the
                # tail z/w chain overlaps the q=1 matmuls.
                for q in range(2):
                    for p in range(2):
                        sl = slice(64 * q, 64 * q + KF)
                        dst = ps3_pair[q][0:32, 32 * p : 32 * p + 32]
                        nc.tensor.matmul(
                            dst,
                            lhsT=Tsb[sl, 64 * p : 64 * p + 32],
                            rhs=CGf1[sl, :],
                            start=True,
                            stop=False,
                            tile_position=(64 * q, 0),
                        )
                        nc.tensor.matmul(
                            dst,
                            lhsT=Tsb[sl, 64 * p + 32 : 64 * p + 64],
                            rhs=CGf2[sl, :],
                            start=False,
                            stop=True,
                            tile_position=(64 * q, 0),
                        )

            def fft2_of(src_grid, npairs=2):
                """Forward FFT2 of a real [64, 128*npairs] grid (k2 folded).
                Returns psum tile [128, 66*npairs] holding (re|im) per pair."""
                ps1 = psA.tile([128, 128 * npairs], dt, tag="psA", name="ps1")
                fft_stage1(src_grid[:], ps1, npairs)
                Hsb = loop.tile([128, 128 * npairs], dt, tag="hsb", name="hsb")
                for p in range(npairs):
                    nc.scalar.copy(
                        Hsb[:, 128 * p : 128 * p + 128],
                        ps1[:, 128 * p : 128 * p + 128],
                    )
                ps2 = psB.tile([128, 66 * npairs], dt, tag="psB", name="ps2")
                fft_stage2(Hsb, ps2, npairs)
                return ps2

            def ifft2_corner(Wsb):
                """Corner of real IFFT2 given folded freq (re|im) SBUF quad.
                Returns (ps3a, ps3b): lanes (0,2) in ps3a, lanes (1,3) in ps3b,
                each [32,64] grid-oriented at cols 0/32."""
                ps1 = psA.tile([128, 128], dt, tag="psA")
                ifft_stage1(Wsb, ps1)
                Tsb = loop.tile([128, 128], dt, tag="tsb")
                # T1 wrote partitions [0,KF) and [64,64+KF) only
                nc.scalar.copy(Tsb[0:KF, :], ps1[0:KF, :])
                nc.scalar.copy(Tsb[64 : 64 + KF, :], ps1[64 : 64 + KF, :])
                ps3a = psC.tile([32, 64], dt, tag="psCa")
                ps3b = psC.tile([32, 64], dt, tag="psCb")
                ifft_stage2(Tsb, (ps3a, ps3b))
                return ps3a, ps3b

            def dot4(a_corner, b_corner, dst_region, tag, prod_eng=None):
                """per-lane dot of two [32,4,32] corner APs -> psum [1,4]"""
                e = prod_eng or nc.vector
                prod = loop.tile([32, 128], dt, tag=f"{tag}_pr", name="prod")
                pview = prod[:].rearrange("p (l c) -> p l c", l=4)
                e.tensor_tensor(out=pview, in0=a_corner, in1=b_corner, op=Alu.mult)
                acc = loop.tile([32, 4], dt, tag=f"{tag}_ac", name="acc")
                nc.vector.tensor_reduce(acc[:], pview, mybir.AxisListType.X, Alu.add)
                nc.tensor.matmul(
                    dst_region, lhsT=ones[0:32, 0:1], rhs=acc[:], start=True,
                    stop=True,
                )

            def bcast4(src_1x4, dst_region):
                # [1,4] sbuf -> [32,4] psum (broadcast along partitions)
                nc.tensor.matmul(
                    dst_region, lhsT=ones[0:1, 0:32], rhs=src_1x4, start=True,
                    stop=True,
                )

            # ---------------- setup: FFT2(X), D, b = Zt d via correlation
            psXD = fft2_of(XDg, npairs=4)
            XDhat = loop.tile([128, 66 * 4], dt, tag="xdhat")
            nc.scalar.copy(XDhat[:], psXD[:])
            Xhat_ap = XDhat[:, 0:132]
            dhat_ap = XDhat[:, 132:264]
            # conj(Xhat) * dhat first (feeds b's IFFT, the critical path);
            # D-build runs on gpsimd in parallel
            Pw = loop.tile([128, 2 * 2 * KF], dt, tag="wsb")
            for p in range(2):
                c0 = slice(66 * p, 66 * p + KF)
                c1 = slice(66 * p + KF, 66 * p + 66)
                t1 = loop.tile([128, KF], dt, tag="cm1")
                t2 = loop.tile([128, KF], dt, tag="cm2")
                nc.vector.tensor_tensor(t1[:], Xhat_ap[:, c0], dhat_ap[:, c0], op=Alu.mult)
                nc.vector.tensor_tensor(t2[:], Xhat_ap[:, c1], dhat_ap[:, c1], op=Alu.mult)
                nc.vector.tensor_tensor(Pw[:, c0], t1[:], t2[:], op=Alu.add)
                t3 = loop.tile([128, KF], dt, tag="cm1")
                t4 = loop.tile([128, KF], dt, tag="cm2")
                nc.vector.tensor_tensor(t3[:], Xhat_ap[:, c0], dhat_ap[:, c1], op=Alu.mult)
                nc.vector.tensor_tensor(t4[:], Xhat_ap[:, c1], dhat_ap[:, c0], op=Alu.mult)
                nc.vector.tensor_tensor(Pw[:, c1], t3[:], t4[:], op=Alu.subtract)

            for p in range(2):
                c0 = slice(66 * p, 66 * p + KF)
                c1 = slice(66 * p + KF, 66 * p + 66)
                nc.gpsimd.tensor_tensor(
                    Dall[:, c0], Xhat_ap[:, c0], Xhat_ap[:, c0], op=Alu.mult
                )
                sq = loop.tile([128, KF], dt, tag="sq")
                nc.gpsimd.tensor_tensor(sq[:], Xhat_ap[:, c1], Xhat_ap[:, c1], op=Alu.mult)
                nc.gpsimd.tensor_tensor(Dall[:, c0], Dall[:, c0], sq[:], op=Alu.add)
                nc.gpsimd.tensor_copy(Dall[:, c1], Dall[:, c0])
            def matvec_raw(src_grid):
                # IFFT2(D .* FFT2(src)), corner psum pair (without +eps*src)
                psP = fft2_of(src_grid)
                Wsb = loop.tile([128, 2 * 2 * KF], dt, tag="wsb")
                nc.vector.tensor_tensor(Wsb[:], psP[:], Dall[:], op=Alu.mult)
                return ifft2_corner(Wsb)

            def matvec(src_grid, dst_grid):
                # dst|corner = IFFT2(D .* FFT2(src))|corner + eps*src
                psqa, psqb = matvec_raw(src_grid)
                for q, psq in ((0, psqa), (1, psqb)):
                    nc.vector.scalar_tensor_tensor(
                        out=corner_q(dst_grid[:], q),
                        in0=corner_q(src_grid[:], q),
                        scalar=float(EPS),
                        in1=psq[:].rearrange("p (l c) -> p l c", l=2),
                        op0=Alu.mult,
                        op1=Alu.add,
                    )

            def axpy(out_c, a_c, coef, b_c, op=Alu.add, tmp_tag="tva",
                     eng=None):
                # out = b_c op (coef .* a_c), coef = sbuf [32,4] per-lane
                e = eng or nc.vector
                cb = coef[:, :, None].broadcast_to([32, 4, 32])
                tmp = loop.tile([32, 128], dt, tag=tmp_tag, name="tmp")
                tv = tmp[:].rearrange("p (l c) -> p l c", l=4)
                e.tensor_tensor(tv, a_c, cb, op=Alu.mult)
                e.tensor_tensor(out_c, b_c, tv, op=op)

            psba, psbb = ifft2_corner(Pw)
            # b -> r0 corner (x0 = 0 so r0 = b)
            for q, psq in ((0, psba), (1, psbb)):
                nc.vector.tensor_copy(
                    corner_q(rv_a[:], q), psq[:].rearrange("p (l c) -> p l c", l=2)
                )
            # w0 = A r0
            matvec(rv_a, wv_a)

            # ---------------- pipelined CG loop (Ghysels-Vanroose)
            r_cur, r_nxt = rv_a, rv_b
            w_cur, w_nxt = wv_a, wv_b
            g_cur, g_prv = gam_t
            a_cur, a_prv = alp_t
            for it in range(ITERS):
                # matvec q = A w  (independent of this iteration's scalars)
                psqa, psqb = matvec_raw(w_cur)

                # dots: gamma = <r,r>, delta = <w,r>  (overlap with matvec).
                # gamma's product runs on gpsimd so it cannot slip ahead of
                # the previous iteration's critical w-update in the DVE queue.
                dot4(corner(r_cur[:]), corner(r_cur[:]), ps_dot[0:1, 0:4], "dg",
                     prod_eng=nc.gpsimd)
                dot4(corner(w_cur[:]), corner(r_cur[:]), ps_dot[0:1, 4:8], "dd")
                gd = loop.tile([1, 8], dt, tag="gd")
                nc.vector.tensor_copy(gd[:], ps_dot[0:1, :])
                nc.vector.tensor_copy(g_cur[:], gd[0:1, 0:4])
                delta = gd[0:1, 4:8]

                if it == 0:
                    # alpha = gamma/delta ; beta = 0
                    rd = loop.tile([1, 4], dt, tag="sc1")
                    nc.vector.reciprocal(rd[:], delta)
                    nc.vector.tensor_tensor(a_cur[:], g_cur[:], rd[:], op=Alu.mult)
                    bcast4(a_cur[0:1, :], ps_bc[0:32, 4:8])
                    # z = q = eps*w + IFFT psum ; s = w ; p = r
                    for q_, psq in ((0, psqa), (1, psqb)):
                        nc.vector.scalar_tensor_tensor(
                            out=corner_q(zv[:], q_),
                            in0=corner_q(w_cur[:], q_),
                            scalar=float(EPS),
                            in1=psq[:].rearrange("p (l c) -> p l c", l=2),
                            op0=Alu.mult,
                            op1=Alu.add,
                        )
                    nc.vector.tensor_copy(corner(sv[:]), corner(w_cur[:]))
                    nc.vector.tensor_copy(corner(pv[:]), corner(r_cur[:]))
                else:
                    # beta = gamma/gamma_prev
                    # alpha = gamma/(delta - beta*gamma/alpha_prev)
                    rgp = loop.tile([1, 4], dt, tag="sc1")
                    nc.vector.reciprocal(rgp[:], g_prv[:])
                    beta = loop.tile([1, 4], dt, tag="sc2")
                    nc.vector.tensor_tensor(beta[:], g_cur[:], rgp[:], op=Alu.mult)
                    rap = loop.tile([1, 4], dt, tag="sc3")
                    nc.vector.reciprocal(rap[:], a_prv[:])
                    t2 = loop.tile([1, 4], dt, tag="sc4")
                    nc.vector.tensor_tensor(t2[:], beta[:], g_cur[:], op=Alu.mult)
                    t3 = loop.tile([1, 4], dt, tag="sc5")
                    nc.vector.tensor_tensor(t3[:], t2[:], rap[:], op=Alu.mult)
                    t4 = loop.tile([1, 4], dt, tag="sc6")
                    nc.vector.tensor_tensor(t4[:], delta, t3[:], op=Alu.subtract)
                    rt4 = loop.tile([1, 4], dt, tag="sc7")
                    nc.vector.reciprocal(rt4[:], t4[:])
                    nc.vector.tensor_tensor(a_cur[:], g_cur[:], rt4[:], op=Alu.mult)
                    bcast4(beta[0:1, :], ps_bc[0:32, 0:4])
                    bcast4(a_cur[0:1, :], ps_bc[0:32, 4:8])
                    if it < ITERS - 1:
                        # z' = (beta z + eps w) + IFFTpsum; the paren part
                        # runs during the matvec, only the psum add is tail
                        tz = loop.tile([32, 128], dt, tag="tvz")
                        tzq = [
                            tz[:].rearrange("p (l c) -> p l c", l=4)[:, q_::2, :]
                            for q_ in range(2)
                        ]
                        bb0 = ps_bc[0:32, 0:4]
                        for q_, psq in ((0, psqa), (1, psqb)):
                            cb = bb0[:, q_::2][:, :, None].broadcast_to([32, 2, 32])
                            nc.vector.tensor_tensor(
                                tzq[q_], corner_q(zv[:], q_), cb, op=Alu.mult
                            )
                            nc.vector.scalar_tensor_tensor(
                                out=tzq[q_], in0=corner_q(w_cur[:], q_),
                                scalar=float(EPS), in1=tzq[q_],
                                op0=Alu.mult, op1=Alu.add,
                            )
                            nc.vector.tensor_tensor(
                                corner_q(zv[:], q_), tzq[q_],
                                psq[:].rearrange("p (l c) -> p l c", l=2),
                                op=Alu.add,
                            )
                        # s = w + beta s
                        axpy(corner(sv[:]), corner(sv[:]), ps_bc[0:32, 0:4],
                             corner(w_cur[:]), tmp_tag="tvs")
                    # p = r + beta p
                    axpy(corner(pv[:]), corner(pv[:]), ps_bc[0:32, 0:4],
                         corner(r_cur[:]), tmp_tag="tvp")

                last = it == ITERS - 1
                if not last:
                    # w' = w - alpha z, per q-half so the q=0 half runs
                    # while T2's q=1 matmuls are still on the PE
                    ab4 = ps_bc[0:32, 4:8]
                    for q_ in range(2):
                        cb = ab4[:, q_::2][:, :, None].broadcast_to([32, 2, 32])
                        tw = loop.tile([32, 64], dt, tag=f"tvw{q_}", name="tw")
                        twv = tw[:].rearrange("p (l c) -> p l c", l=2)
                        nc.vector.tensor_tensor(
                            twv, corner_q(zv[:], q_), cb, op=Alu.mult
                        )
                        nc.vector.tensor_tensor(
                            corner_q(w_nxt[:], q_), corner_q(w_cur[:], q_),
                            twv, op=Alu.subtract,
                        )
                    # r' = r - alpha s
                    axpy(corner(r_nxt[:]), corner(sv[:]), ps_bc[0:32, 4:8],
                         corner(r_cur[:]), op=Alu.subtract, tmp_tag="tvr")
                # x += alpha p (off critical path; add on gpsimd)
                tmpx = loop.tile([32, 128], dt, tag="tvx")
                txv = tmpx[:].rearrange("p (l c) -> p l c", l=4)
                ab = ps_bc[0:32, 4:8][:, :, None].broadcast_to([32, 4, 32])
                nc.vector.tensor_tensor(txv, corner(pv[:]), ab, op=Alu.mult)
                nc.gpsimd.tensor_tensor(corner(xv[:]), corner(xv[:]), txv, op=Alu.add)

                r_cur, r_nxt = r_nxt, r_cur
                w_cur, w_nxt = w_nxt, w_cur
                g_cur, g_prv = g_prv, g_cur
                a_cur, a_prv = a_prv, a_cur

            # ---------------- epilogue: losses = 0.5*||T.*w||/||w||
            wt = loop.tile([32, 128], dt, tag="tmpx")
            wtv = wt[:].rearrange("p (l c) -> p l c", l=4)
            tb = Tq[:, None, :].broadcast_to([32, 4, 32])
            nc.vector.tensor_tensor(wtv, corner(xv[:]), tb, op=Alu.mult)
            dot4(wtv, wtv, ps_dot[0:1, 0:4], "en1", prod_eng=nc.vector)
            dot4(corner(xv[:]), corner(xv[:]), ps_dot[0:1, 4:8], "en2",
                 prod_eng=nc.vector)
            ns = loop.tile([1, 8], dt, tag="ns")
            nc.vector.tensor_copy(ns[:], ps_dot[0:1, :])
            ns2 = loop.tile([1, 8], dt, tag="ns2")
            nc.scalar.sqrt(ns2[:], ns[:])
            dinv = loop.tile([1, 4], dt, tag="sc1")
            nc.vector.reciprocal(dinv[:], ns2[0:1, 4:8])
            ratio = loop.tile([1, 4], dt, tag="sc2")
            nc.vector.tensor_tensor(ratio[:], ns2[0:1, 0:4], dinv[:], op=Alu.mult)
            loss_sb = loop.tile([1, 4], dt, tag="sc3")
            nc.scalar.mul(loss_sb[:], ratio[:], 0.5)
            nc.sync.dma_start(out, loss_sb[:])

    return nc


def get_nc():
    if "nc" not in _NC_CACHE:
        nc = build_nc()
        if not nc.is_finalized():
            nc.finalize()
        _NC_CACHE["nc"] = nc
    return _NC_CACHE["nc"]


# ---------------------------------------------------------------- entry point
def kernel(recon: np.ndarray, target: np.ndarray) -> np.ndarray:
    from concourse.bass_utils import run_bass_kernel_spmd

    rec = np.ascontiguousarray(np.asarray(recon, dtype=F32).reshape(24, H, W))
    tgt = np.ascontiguousarray(np.asarray(target, dtype=F32).reshape(24, H, W))

    in_maps = []
    for c in range(N_CORES):
        idx = [3 * c, 3 * c + 1, 3 * c + 2, 3 * c + 2]  # lane 3 duplicates
        in_maps.append({"tq": tgt[idx].copy(), "rq": rec[idx].copy()})

    nc = get_nc()
    res = run_bass_kernel_spmd(nc, in_maps, list(range(N_CORES)))
    total = F32(0.0)
    for c in range(N_CORES):
        total += res.results[c]["loss"][0, :3].astype(F32).sum(dtype=F32)
    return np.asarray(total, dtype=F32)



# revision 2
# speedup vs baseline: 2.2392x; 2.2392x over previous
"""AWLoss2D Trainium2 kernel.

Math summary (per sample, H=W=32):
  Z = full-2D-conv (doubly-blocked Toeplitz) matrix of target patch X, [3969, 1024]
  v = Z^T Z + eps*I  (BTTB, diagonalized-modulo-support by the 64x64 DFT)
  w = v^{-1} Z^T d   (d = centered zero-pad of recon patch)
  loss = 0.5*||T2D .* w|| / ||w||, summed over 24 samples.

Device algorithm: pipelined CG (Ghysels-Vanroose, fixed 12 iters) on
v w = b, where the matvec
  v p = P^T IFFT2( |FFT2(X)|^2 .* FFT2(P p) ) + eps p
is computed with explicit 64x64 DFT-matrix matmuls on the tensor engine
(F is symmetric, so stage S(G) := G^T F applied twice gives FFT2 with no
transposes; real-input Hermitian symmetry folds the column-frequency axis
to 33 bins and the inverse stages produce only the 32x32 corner).
24 samples = 8 cores x 3 lanes (4th lane duplicates the 3rd).
fp32 PIPECG at 12 iters vs f64 reference: total rel err ~7e-5.
"""

import numpy as np

H = W = 32
N = 64  # FFT grid
LANES = 4  # per core: 3 real samples + 1 duplicate
N_CORES = 8
ITERS = 3
EPS = 1e-4
F32 = np.float32

_NC_CACHE = {}


# ---------------------------------------------------------------- host consts
def _t2d_flat():
    xarr = np.linspace(-10.0, 10.0, H)
    yarr = np.linspace(-10.0, 10.0, W)
    xx, yy = np.meshgrid(xarr, yarr, indexing="ij")
    dispx = (H % 2 - 1) / 2.0
    dispy = (W % 2 - 1) / 2.0
    dx = (xarr[-1] - xarr[0]) / (H - 1)
    dy = (yarr[-1] - yarr[0]) / (W - 1)
    sx = sy = 1.0
    t = -(1.0 / (2.0 * np.pi * sx * sy)) * np.exp(
        -((xx - dx * dispx) ** 2 / (2 * sx**2) + (yy - dy * dispy) ** 2 / (2 * sy**2))
    )
    t = t + np.max(np.abs(t))
    return (t / np.max(np.abs(t))).astype(F32)


KF = 33  # folded frequency count (Hermitian symmetry of real-input FFT)


def _dft_consts():
    k = np.arange(N)
    Fc = np.exp(-2j * np.pi * np.outer(k, k) / N)  # symmetric
    Fr = Fc.real
    Fi = Fc.imag
    Gr = Fc.real / N       # conj(F)/N, real part
    Gi = -Fc.imag / N      # conj(F)/N, imag part
    # S1 rhs: full [Fr | Fi] (all 64 row-freqs k)
    CF = np.concatenate([Fr, Fi], axis=1)                      # [64,128]
    # S2 rhs: column-freqs folded to k2 in [0,33)
    CFh = np.concatenate([Fr[:, :KF], Fi[:, :KF]], axis=1)     # [64,66]
    CF2h = np.concatenate([-Fi[:, :KF], Fr[:, :KF]], axis=1)
    # T1 rhs: inverse over row-freq k (full 64), spatial rows r in [0,32) only
    CG1h = np.concatenate([Gr[:, :32], Gi[:, :32]], axis=1)    # [64,64]
    CG2h = np.concatenate([-Gi[:, :32], Gr[:, :32]], axis=1)
    # T2 rhs: inverse over folded k2 with Hermitian weights, cols c in [0,32)
    w = np.ones((KF, 1))
    w[1:32] = 2.0  # k2 = 0 and 32 are self-paired
    CGf1 = np.zeros((64, 32))
    CGf2 = np.zeros((64, 32))
    CGf1[:KF] = w * Gr[:KF, :32]
    CGf2[:KF] = w * (-Gi[:KF, :32])
    dup = lambda a: np.concatenate([a, a], axis=0).astype(F32)
    return tuple(dup(a) for a in (CF, CFh, CF2h, CG1h, CG2h, CGf1, CGf2))


# ---------------------------------------------------------------- bass program
def build_nc():
    import concourse.bass as bass
    import concourse.mybir as mybir
    import concourse.tile as tile
    from concourse import bacc

    dt = mybir.dt.float32
    Alu = mybir.AluOpType

    nc = bacc.Bacc("TRN2", target_bir_lowering=False)

    tq = nc.dram_tensor("tq", [LANES, H, W], dt, kind="ExternalInput").ap()
    rq = nc.dram_tensor("rq", [LANES, H, W], dt, kind="ExternalInput").ap()
    out = nc.dram_tensor("loss", [1, LANES], dt, kind="ExternalOutput").ap()

    const_np = _dft_consts()
    const_names = ("cf", "cfh", "cf2h", "cg1h", "cg2h", "cgf1", "cgf2")
    # merge every constant into one [128, sum] block -> single DMA at start
    t2d128 = np.zeros((128, 32), dtype=F32)
    t2d128[0:32] = _t2d_flat()
    all_consts = np.concatenate(list(const_np) + [t2d128], axis=1)
    cwidths = [a.shape[1] for a in const_np] + [32]
    allc_d = nc.inline_tensor(all_consts, "allc").ap()

    def corner(ap32wide, blk=N, w=32):
        # [32, nblk, w] view of the first 32 partitions, first w cols per block
        nblk = ap32wide.shape[1] // blk
        return ap32wide[0:32].rearrange("p (l c) -> p l c", l=nblk)[:, :, 0:w]

    def corner_q(grid, q):
        # lanes {q, q+2} corners of a [64,256] grid: [32, 2, 32]
        return grid[0:32].rearrange("p (l c) -> p l c", l=4)[:, q::2, 0:32]

    with tile.TileContext(nc) as tc:
        with (
            tc.tile_pool(name="consts", bufs=1) as consts,
            tc.tile_pool(name="state", bufs=1) as state,
            tc.tile_pool(name="loop", bufs=2) as loop,
            tc.tile_pool(name="psA", bufs=2, space="PSUM") as psA,
            tc.tile_pool(name="psB", bufs=2, space="PSUM") as psB,
            tc.tile_pool(name="psC", bufs=1, space="PSUM") as psC,
            tc.tile_pool(name="psS", bufs=1, space="PSUM") as psS,
        ):
            # ---------------- constants into SBUF (one DMA)
            CALL = consts.tile([128, all_consts.shape[1]], dt)
            nc.sync.dma_start(CALL[:], allc_d)
            offs = np.cumsum([0] + cwidths)
            cslices = [CALL[:, int(offs[i]) : int(offs[i + 1])]
                       for i in range(len(cwidths))]
            CF, CFh, CF2h, CG1h, CG2h, CGf1, CGf2, TqF = cslices
            Tq = TqF[0:32, :]
            ones = consts.tile([64, 64], dt)
            nc.any.memset(ones[:], 1.0)

            # ---------------- persistent state
            XDg = state.tile([64, 512], dt)    # target grids (lanes 0-3) +
                                               # padded recon grids (lanes 4-7)
            Dall = state.tile([128, 2 * 2 * KF], dt)  # |FFT2 X|^2 dup, per pair
            xv = state.tile([64, 256], dt)
            rv_a = state.tile([64, 256], dt)
            rv_b = state.tile([64, 256], dt)
            wv_a = state.tile([64, 256], dt)
            wv_b = state.tile([64, 256], dt)
            pv = state.tile([64, 256], dt)
            sv = state.tile([64, 256], dt)
            zv = state.tile([64, 256], dt)
            gam_t = [
                state.tile([1, 4], dt, tag=f"gam{i}", name=f"gam{i}")
                for i in range(2)
            ]
            alp_t = [
                state.tile([1, 4], dt, tag=f"alp{i}", name=f"alp{i}")
                for i in range(2)
            ]
            ps_dot = psS.tile([1, 8], dt, tag="sdot")    # gamma | delta
            ps_bc = psS.tile([32, 8], dt, tag="sbc")     # beta | alpha bcast

            for t in (XDg, xv, rv_a, rv_b, wv_a, wv_b, pv, sv, zv):
                nc.any.memset(t[:], 0.0)

            # load inputs into grid corners
            nc.sync.dma_start(
                corner(XDg[:, 0:256]), tq.rearrange("l h w -> h l w")
            )
            nc.sync.dma_start(
                XDg[15:47, 256:512].rearrange("p (l c) -> p l c", l=4)[:, :, 15:47],
                rq.rearrange("l h w -> h l w"),
            )

            # ---------------- helpers
            def fft_stage1(src, ps, npairs=2):
                # S1: for pair p: out[:, 128p:128p+128] = src_pair^T @ [Fr|Fi]
                for p in range(npairs):
                    nc.tensor.matmul(
                        ps[:, 128 * p : 128 * p + 128],
                        lhsT=src[:, 128 * p : 128 * p + 128],
                        rhs=CF[0:64, :],
                        start=True,
                        stop=True,
                    )

            def fft_stage2(Hsb, ps, npairs=2):
                # S2: per pair p, per half q (lane 2p+q at partitions 64q):
                #   out = Hre^T [Fr|Fi] + Him^T [-Fi|Fr], col-freqs folded to KF
                for p in range(npairs):
                    for q in range(2):
                        sl = slice(64 * q, 64 * q + 64)
                        dst = ps[sl, 66 * p : 66 * p + 66]
                        nc.tensor.matmul(
                            dst,
                            lhsT=Hsb[sl, 128 * p : 128 * p + 64],
                            rhs=CFh[sl, :],
                            start=True,
                            stop=False,
                        )
                        nc.tensor.matmul(
                            dst,
                            lhsT=Hsb[sl, 128 * p + 64 : 128 * p + 128],
                            rhs=CF2h[sl, :],
                            start=False,
                            stop=True,
                        )

            def ifft_stage1(Wsb, ps):
                # T1: out[k2, r] for k2 in [0,KF), r in [0,32):
                #   (T1re|T1im) = Wre^T [Gr|Gi] + Wim^T [-Gi|Gr]
                for p in range(2):
                    for q in range(2):
                        sl = slice(64 * q, 64 * q + 64)
                        dst = ps[64 * q : 64 * q + KF, 64 * p : 64 * p + 64]
                        nc.tensor.matmul(
                            dst,
                            lhsT=Wsb[sl, 66 * p : 66 * p + KF],
                            rhs=CG1h[sl, :],
                            start=True,
                            stop=False,
                        )
                        nc.tensor.matmul(
                            dst,
                            lhsT=Wsb[sl, 66 * p + KF : 66 * p + 66],
                            rhs=CG2h[sl, :],
                            start=False,
                            stop=True,
                        )

            def ifft_stage2(Tsb, ps3_pair):
                # T2 (real part, 32x32 corner, Hermitian-weighted fold over
                # k2 in [0,KF)): lane l=2p+q -> ps3_pair[q][0:32, 32p:32p+32].
                # q=0/q=1 use different row groups; they must land in
                # different PSUM tiles (mixed tile_position into one PSUM
                # bank faults).  q-outer: lanes (0,2) finish first so the
                # tail z/w chain overlaps the q=1 matmuls.
                for q in range(2):
                    for p in range(2):
                        sl = slice(64 * q, 64 * q + KF)
                        dst = ps3_pair[q][0:32, 32 * p : 32 * p + 32]
                        nc.tensor.matmul(
                            dst,
                            lhsT=Tsb[sl, 64 * p : 64 * p + 32],
                            rhs=CGf1[sl, :],
                            start=True,
                            stop=False,
                            tile_position=(64 * q, 0),
                        )
                        nc.tensor.matmul(
                            dst,
                            lhsT=Tsb[sl, 64 * p + 32 : 64 * p + 64],
                            rhs=CGf2[sl, :],
                            start=False,
                            stop=True,
                            tile_position=(64 * q, 0),
                        )

            def fft2_of(src_grid, npairs=2):
                """Forward FFT2 of a real [64, 128*npairs] grid (k2 folded).
                Returns psum tile [128, 66*npairs] holding (re|im) per pair."""
                ps1 = psA.tile([128, 128 * npairs], dt, tag="psA", name="ps1")
                fft_stage1(src_grid[:], ps1, npairs)
                Hsb = loop.tile([128, 128 * npairs], dt, tag="hsb", name="hsb")
                for p in range(npairs):
                    nc.scalar.copy(
                        Hsb[:, 128 * p : 128 * p + 128],
                        ps1[:, 128 * p : 128 * p + 128],
                    )
                ps2 = psB.tile([128, 66 * npairs], dt, tag="psB", name="ps2")
                fft_stage2(Hsb, ps2, npairs)
                return ps2

            def ifft2_corner(Wsb):
                """Corner of real IFFT2 given folded freq (re|im) SBUF quad.
                Returns (ps3a, ps3b): lanes (0,2) in ps3a, lanes (1,3) in ps3b,
                each [32,64] grid-oriented at cols 0/32."""
                ps1 = psA.tile([128, 128], dt, tag="psA")
                ifft_stage1(Wsb, ps1)
                Tsb = loop.tile([128, 128], dt, tag="tsb")
                # T1 wrote partitions [0,KF) and [64,64+KF) only
                nc.scalar.copy(Tsb[0:KF, :], ps1[0:KF, :])
                nc.scalar.copy(Tsb[64 : 64 + KF, :], ps1[64 : 64 + KF, :])
                ps3a = psC.tile([32, 64], dt, tag="psCa")
                ps3b = psC.tile([32, 64], dt, tag="psCb")
                ifft_stage2(Tsb, (ps3a, ps3b))
                return ps3a, ps3b

            def dot4(a_corner, b_corner, dst_region, tag, prod_eng=None):
                """per-lane dot of two [32,4,32] corner APs -> psum [1,4]"""
                e = prod_eng or nc.vector
                prod = loop.tile([32, 128], dt, tag=f"{tag}_pr", name="prod")
                pview = prod[:].rearrange("p (l c) -> p l c", l=4)
                e.tensor_tensor(out=pview, in0=a_corner, in1=b_corner, op=Alu.mult)
                acc = loop.tile([32, 4], dt, tag=f"{tag}_ac", name="acc")
                nc.vector.tensor_reduce(acc[:], pview, mybir.AxisListType.X, Alu.add)
                nc.tensor.matmul(
                    dst_region, lhsT=ones[0:32, 0:1], rhs=acc[:], start=True,
                    stop=True,
                )

            def bcast4(src_1x4, dst_region):
                # [1,4] sbuf -> [32,4] psum (broadcast along partitions)
                nc.tensor.matmul(
                    dst_region, lhsT=ones[0:1, 0:32], rhs=src_1x4, start=True,
                    stop=True,
                )

            # ---------------- setup: FFT2(X), D, b = Zt d via correlation
            psXD = fft2_of(XDg, npairs=4)
            XDhat = loop.tile([128, 66 * 4], dt, tag="xdhat")
            nc.scalar.copy(XDhat[:], psXD[:])
            Xhat_ap = XDhat[:, 0:132]
            dhat_ap = XDhat[:, 132:264]
            # conj(Xhat) * dhat first (feeds b's IFFT, the critical path);
            # D-build runs on gpsimd in parallel
            Pw = loop.tile([128, 2 * 2 * KF], dt, tag="wsb")
            for p in range(2):
                c0 = slice(66 * p, 66 * p + KF)
                c1 = slice(66 * p + KF, 66 * p + 66)
                t1 = loop.tile([128, KF], dt, tag="cm1")
                t2 = loop.tile([128, KF], dt, tag="cm2")
                nc.vector.tensor_tensor(t1[:], Xhat_ap[:, c0], dhat_ap[:, c0], op=Alu.mult)
                nc.vector.tensor_tensor(t2[:], Xhat_ap[:, c1], dhat_ap[:, c1], op=Alu.mult)
                nc.vector.tensor_tensor(Pw[:, c0], t1[:], t2[:], op=Alu.add)
                t3 = loop.tile([128, KF], dt, tag="cm1")
                t4 = loop.tile([128, KF], dt, tag="cm2")
                nc.vector.tensor_tensor(t3[:], Xhat_ap[:, c0], dhat_ap[:, c1], op=Alu.mult)
                nc.vector.tensor_tensor(t4[:], Xhat_ap[:, c1], dhat_ap[:, c0], op=Alu.mult)
                nc.vector.tensor_tensor(Pw[:, c1], t3[:], t4[:], op=Alu.subtract)

            for p in range(2):
                c0 = slice(66 * p, 66 * p + KF)
                c1 = slice(66 * p + KF, 66 * p + 66)
                nc.gpsimd.tensor_tensor(
                    Dall[:, c0], Xhat_ap[:, c0], Xhat_ap[:, c0], op=Alu.mult
                )
                sq = loop.tile([128, KF], dt, tag="sq")
                nc.gpsimd.tensor_tensor(sq[:], Xhat_ap[:, c1], Xhat_ap[:, c1], op=Alu.mult)
                nc.gpsimd.tensor_tensor(Dall[:, c0], Dall[:, c0], sq[:], op=Alu.add)
                nc.gpsimd.tensor_copy(Dall[:, c1], Dall[:, c0])
            def matvec_raw(src_grid):
                # IFFT2(D .* FFT2(src)), corner psum pair (without +eps*src)
                psP = fft2_of(src_grid)
                Wsb = loop.tile([128, 2 * 2 * KF], dt, tag="wsb")
                nc.vector.tensor_tensor(Wsb[:], psP[:], Dall[:], op=Alu.mult)
                return ifft2_corner(Wsb)

            def matvec(src_grid, dst_grid):
                # dst|corner = IFFT2(D .* FFT2(src))|corner + eps*src
                psqa, psqb = matvec_raw(src_grid)
                for q, psq in ((0, psqa), (1, psqb)):
                    nc.vector.scalar_tensor_tensor(
                        out=corner_q(dst_grid[:], q),
                        in0=corner_q(src_grid[:], q),
                        scalar=float(EPS),
                        in1=psq[:].rearrange("p (l c) -> p l c", l=2),
                        op0=Alu.mult,
                        op1=Alu.add,
                    )

            def axpy(out_c, a_c, coef, b_c, op=Alu.add, tmp_tag="tva",
                     eng=None):
                # out = b_c op (coef .* a_c), coef = sbuf [32,4] per-lane
                e = eng or nc.vector
                cb = coef[:, :, None].broadcast_to([32, 4, 32])
                tmp = loop.tile([32, 128], dt, tag=tmp_tag, name="tmp")
                tv = tmp[:].rearrange("p (l c) -> p l c", l=4)
                e.tensor_tensor(tv, a_c, cb, op=Alu.mult)
                e.tensor_tensor(out_c, b_c, tv, op=op)

            psba, psbb = ifft2_corner(Pw)
            # b -> r0 corner (x0 = 0 so r0 = b)
            for q, psq in ((0, psba), (1, psbb)):
                nc.vector.tensor_copy(
                    corner_q(rv_a[:], q), psq[:].rearrange("p (l c) -> p l c", l=2)
                )
            # w0 = A r0
            matvec(rv_a, wv_a)

            # ---------------- pipelined CG loop (Ghysels-Vanroose)
            r_cur, r_nxt = rv_a, rv_b
            w_cur, w_nxt = wv_a, wv_b
            g_cur, g_prv = gam_t
            a_cur, a_prv = alp_t
            for it in range(ITERS):
                # matvec q = A w  (independent of this iteration's scalars)
                psqa, psqb = matvec_raw(w_cur)

                # dots: gamma = <r,r>, delta = <w,r>  (overlap with matvec).
                # gamma's product runs on gpsimd so it cannot slip ahead of
                # the previous iteration's critical w-update in the DVE queue.
                dot4(corner(r_cur[:]), corner(r_cur[:]), ps_dot[0:1, 0:4], "dg",
                     prod_eng=nc.gpsimd)
                dot4(corner(w_cur[:]), corner(r_cur[:]), ps_dot[0:1, 4:8], "dd")
                gd = loop.tile([1, 8], dt, tag="gd")
                nc.vector.tensor_copy(gd[:], ps_dot[0:1, :])
                nc.vector.tensor_copy(g_cur[:], gd[0:1, 0:4])
                delta = gd[0:1, 4:8]

                if it == 0:
                    # alpha = gamma/delta ; beta = 0
                    rd = loop.tile([1, 4], dt, tag="sc1")
                    nc.vector.reciprocal(rd[:], delta)
                    nc.vector.tensor_tensor(a_cur[:], g_cur[:], rd[:], op=Alu.mult)
                    bcast4(a_cur[0:1, :], ps_bc[0:32, 4:8])
                    # z = q = eps*w + IFFT psum ; s = w ; p = r
                    for q_, psq in ((0, psqa), (1, psqb)):
                        nc.vector.scalar_tensor_tensor(
                            out=corner_q(zv[:], q_),
                            in0=corner_q(w_cur[:], q_),
                            scalar=float(EPS),
                            in1=psq[:].rearrange("p (l c) -> p l c", l=2),
                            op0=Alu.mult,
                            op1=Alu.add,
                        )
                    nc.vector.tensor_copy(corner(sv[:]), corner(w_cur[:]))
                    nc.vector.tensor_copy(corner(pv[:]), corner(r_cur[:]))
                else:
                    # beta = gamma/gamma_prev
                    # alpha = gamma/(delta - beta*gamma/alpha_prev)
                    rgp = loop.tile([1, 4], dt, tag="sc1")
                    nc.vector.reciprocal(rgp[:], g_prv[:])
                    beta = loop.tile([1, 4], dt, tag="sc2")
                    nc.vector.tensor_tensor(beta[:], g_cur[:], rgp[:], op=Alu.mult)
                    rap = loop.tile([1, 4], dt, tag="sc3")
                    nc.vector.reciprocal(rap[:], a_prv[:])
                    t2 = loop.tile([1, 4], dt, tag="sc4")
                    nc.vector.tensor_tensor(t2[:], beta[:], g_cur[:], op=Alu.mult)
                    t3 = loop.tile([1, 4], dt, tag="sc5")
                    nc.vector.tensor_tensor(t3[:], t2[:], rap[:], op=Alu.mult)
                    t4 = loop.tile([1, 4], dt, tag="sc6")
                    nc.vector.tensor_tensor(t4[:], delta, t3[:], op=Alu.subtract)
                    rt4 = loop.tile([1, 4], dt, tag="sc7")
                    nc.vector.reciprocal(rt4[:], t4[:])
                    nc.vector.tensor_tensor(a_cur[:], g_cur[:], rt4[:], op=Alu.mult)
                    bcast4(beta[0:1, :], ps_bc[0:32, 0:4])
                    bcast4(a_cur[0:1, :], ps_bc[0:32, 4:8])
                    if it < ITERS - 1:
                        # z' = (beta z + eps w) + IFFTpsum; the paren part
                        # runs during the matvec, only the psum add is tail
                        tz = loop.tile([32, 128], dt, tag="tvz")
                        tzq = [
                            tz[:].rearrange("p (l c) -> p l c", l=4)[:, q_::2, :]
                            for q_ in range(2)
                        ]
                        bb0 = ps_bc[0:32, 0:4]
                        for q_, psq in ((0, psqa), (1, psqb)):
                            cb = bb0[:, q_::2][:, :, None].broadcast_to([32, 2, 32])
                            nc.vector.tensor_tensor(
                                tzq[q_], corner_q(zv[:], q_), cb, op=Alu.mult
                            )
                            nc.vector.scalar_tensor_tensor(
                                out=tzq[q_], in0=corner_q(w_cur[:], q_),
                                scalar=float(EPS), in1=tzq[q_],
                                op0=Alu.mult, op1=Alu.add,
                            )
                            nc.vector.tensor_tensor(
                                corner_q(zv[:], q_), tzq[q_],
                                psq[:].rearrange("p (l c) -> p l c", l=2),
                                op=Alu.add,
                            )
                        # s = w + beta s
                        axpy(corner(sv[:]), corner(sv[:]), ps_bc[0:32, 0:4],
                             corner(w_cur[:]), tmp_tag="tvs")
                    # p = r + beta p
                    axpy(corner(pv[:]), corner(pv[:]), ps_bc[0:32, 0:4],
                         corner(r_cur[:]), tmp_tag="tvp")

                last = it == ITERS - 1
                if not last:
                    # w' = w - alpha z, per q-half so the q=0 half runs
                    # while T2's q=1 matmuls are still on the PE
                    ab4 = ps_bc[0:32, 4:8]
                    for q_ in range(2):
                        cb = ab4[:, q_::2][:, :, None].broadcast_to([32, 2, 32])
                        tw = loop.tile([32, 64], dt, tag=f"tvw{q_}", name="tw")
                        twv = tw[:].rearrange("p (l c) -> p l c", l=2)
                        nc.vector.tensor_tensor(
                            twv, corner_q(zv[:], q_), cb, op=Alu.mult
                        )
                        nc.vector.tensor_tensor(
                            corner_q(w_nxt[:], q_), corner_q(w_cur[:], q_),
                            twv, op=Alu.subtract,
                        )
                    # r' = r - alpha s
                    axpy(corner(r_nxt[:]), corner(sv[:]), ps_bc[0:32, 4:8],
                         corner(r_cur[:]), op=Alu.subtract, tmp_tag="tvr")
                # x += alpha p (off critical path; add on gpsimd)
                tmpx = loop.tile([32, 128], dt, tag="tvx")
                txv = tmpx[:].rearrange("p (l c) -> p l c", l=4)
                ab = ps_bc[0:32, 4:8][:, :, None].broadcast_to([32, 4, 32])
                nc.vector.tensor_tensor(txv, corner(pv[:]), ab, op=Alu.mult)
                nc.gpsimd.tensor_tensor(corner(xv[:]), corner(xv[:]), txv, op=Alu.add)

                r_cur, r_nxt = r_nxt, r_cur
                w_cur, w_nxt = w_nxt, w_cur
                g_cur, g_prv = g_prv, g_cur
                a_cur, a_prv = a_prv, a_cur

            # ---------------- epilogue: losses = 0.5*||T.*w||/||w||
            wt = loop.tile([32, 128], dt, tag="tmpx")
            wtv = wt[:].rearrange("p (l c) -> p l c", l=4)
            tb = Tq[:, None, :].broadcast_to([32, 4, 32])
            nc.vector.tensor_tensor(wtv, corner(xv[:]), tb, op=Alu.mult)
            dot4(wtv, wtv, ps_dot[0:1, 0:4], "en1", prod_eng=nc.vector)
            dot4(corner(xv[:]), corner(xv[:]), ps_dot[0:1, 4:8], "en2",
                 prod_eng=nc.vector)
            ns = loop.tile([1, 8], dt, tag="ns")
            nc.vector.tensor_copy(ns[:], ps_dot[0:1, :])
            ns2 = loop.tile([1, 8], dt, tag="ns2")
            nc.scalar.sqrt(ns2[:], ns[:])
            dinv = loop.tile([1, 4], dt, tag="sc1")
            nc.vector.reciprocal(dinv[:], ns2[0:1, 4:8])
            ratio = loop.tile([1, 4], dt, tag="sc2")
            nc.vector.tensor_tensor(ratio[:], ns2[0:1, 0:4], dinv[:], op=Alu.mult)
            loss_sb = loop.tile([1, 4], dt, tag="sc3")
            nc.scalar.mul(loss_sb[:], ratio[:], 0.5)
            nc.sync.dma_start(out, loss_sb[:])

    return nc


def get_nc():
    if "nc" not in _NC_CACHE:
        nc = build_nc()
        if not nc.is_finalized():
            nc.finalize()
        _NC_CACHE["nc"] = nc
    return _NC_CACHE["nc"]


# ---------------------------------------------------------------- entry point
def kernel(recon: np.ndarray, target: np.ndarray) -> np.ndarray:
    from concourse.bass_utils import run_bass_kernel_spmd

    rec = np.ascontiguousarray(np.asarray(recon, dtype=F32).reshape(24, H, W))
    tgt = np.ascontiguousarray(np.asarray(target, dtype=F32).reshape(24, H, W))

    in_maps = []
    for c in range(N_CORES):
        idx = [3 * c, 3 * c + 1, 3 * c + 2, 3 * c + 2]  # lane 3 duplicates
        in_maps.append({"tq": tgt[idx].copy(), "rq": rec[idx].copy()})

    nc = get_nc()
    res = run_bass_kernel_spmd(nc, in_maps, list(range(N_CORES)))
    total = F32(0.0)
    for c in range(N_CORES):
        total += res.results[c]["loss"][0, :3].astype(F32).sum(dtype=F32)
    return np.asarray(total, dtype=F32)



# revision 8
# speedup vs baseline: 2.5814x; 1.1528x over previous
"""AWLoss2D Trainium2 kernel (v2: fp16 tensor path, merged block-diag DFTs).

Math per sample (H=W=32): Z = full-conv Toeplitz of target X [3969,1024];
v = Z^T Z + eps I; w = v^{-1} Z^T d (d = centered zero-pad of recon);
loss = 0.5*||T2D .* w|| / ||w||, summed over 24 samples.

Device algorithm: Ghysels-Vanroose pipelined CG (ITERS fixed) on v w = b
with the BTTB matvec v p = P^T IFFT2(|FFT2 X|^2 .* FFT2(P p)) + eps p
computed by explicit 64-pt DFT matmuls on the tensor engine.

v2 layout: 4 lanes (3 samples + 1 dup) per core live in ONE [128,128]
"quad" grid: partition = (b, row r<64), free = (q, col c<64), lane=2b+q.
Every DFT stage is a K=128 matmul with block-diagonal constants:
  S1 [128,128,256] -> S2 4x[128,64,132] (col-freqs folded to 33 by
  Hermitian symmetry) -> D mult -> T1 2x[128,66,256] -> T2 2x[66,128,128].
T1/T2 constants are zero-padded outside the 32x32 corner, so the IFFT
output lands as a full zero-padded quad grid: the P P^T projection is
free. All matmul operands fp16 (PSUM accumulates fp32; CG truncation at
3 iters dominates the error: total rel err ~2e-3 vs the 2e-2 gate).
A 2^-6 scale folded into the S1 constants keeps every intermediate in
fp16 range (the operator/rhs scaling cancels in the norm-ratio loss).
"""

import numpy as np

H = W = 32
N = 64   # FFT grid
KF = 33  # folded col-freq count
N_CORES = 8
ITERS = 3
EPS = 1e-4
SC = 2.0 ** -6           # scale folded into S1 consts
F32 = np.float32
F16 = np.float16

_NC_CACHE = {}


# ---------------------------------------------------------------- host consts
def _t2d_half():
    xarr = np.linspace(-10.0, 10.0, H)
    yarr = np.linspace(-10.0, 10.0, W)
    xx, yy = np.meshgrid(xarr, yarr, indexing="ij")
    dispx = dispy = (H % 2 - 1) / 2.0
    dx = (xarr[-1] - xarr[0]) / (H - 1)
    dy = (yarr[-1] - yarr[0]) / (W - 1)
    t = -(1.0 / (2.0 * np.pi)) * np.exp(
        -((xx - dx * dispx) ** 2 / 2 + (yy - dy * dispy) ** 2 / 2))
    t = t + np.max(np.abs(t))
    return (0.5 * t / np.max(np.abs(t))).astype(F32)  # 0.5 loss factor folded


def _consts():
    k = np.arange(N)
    Fc = np.exp(-2j * np.pi * np.outer(k, k) / N)
    Fr = Fc.real.astype(F32)
    Fi = Fc.imag.astype(F32)
    Gr = (Fc.real / N).astype(F32)
    Gi = (-Fc.imag / N).astype(F32)  # conj(F)/N

    CF2 = np.zeros((128, 256), F32)   # S1: rows (b,r), cols (b, ri, k1)
    for b in range(2):
        CF2[64 * b:64 * b + 64, 128 * b:128 * b + 64] = SC * Fr
        CF2[64 * b:64 * b + 64, 128 * b + 64:128 * b + 128] = SC * Fi

    CFh2a = np.zeros((128, 132), F32)  # S2 lhsT=Hre: rows (q,c), cols (q,ri,k2f)
    CFh2b = np.zeros((128, 132), F32)  # S2 lhsT=Him
    for q in range(2):
        r0, c0 = 64 * q, 66 * q
        CFh2a[r0:r0 + 64, c0:c0 + KF] = Fr[:, :KF]
        CFh2a[r0:r0 + 64, c0 + KF:c0 + 66] = Fi[:, :KF]
        CFh2b[r0:r0 + 64, c0:c0 + KF] = -Fi[:, :KF]
        CFh2b[r0:r0 + 64, c0 + KF:c0 + 66] = Fr[:, :KF]

    Grp = np.zeros((N, N), F32)        # r-padded inverse row consts
    Gip = np.zeros((N, N), F32)
    Grp[:, :32] = Gr[:, :32]
    Gip[:, :32] = Gi[:, :32]
    CT1a = np.zeros((128, 256), F32)   # T1 lhsT=Wre: rows (b,k1), cols (ri,b,r)
    CT1b = np.zeros((128, 256), F32)   # T1 lhsT=Wim
    for b in range(2):
        r0 = 64 * b
        CT1a[r0:r0 + 64, 64 * b:64 * b + 64] = Grp
        CT1a[r0:r0 + 64, 128 + 64 * b:128 + 64 * b + 64] = Gip
        CT1b[r0:r0 + 64, 64 * b:64 * b + 64] = -Gip
        CT1b[r0:r0 + 64, 128 + 64 * b:128 + 64 * b + 64] = Grp

    wH = np.ones((KF, 1), F32)
    wH[1:32] = 2.0                     # Hermitian fold weights
    wGr = np.zeros((KF, N), F32)
    wGi = np.zeros((KF, N), F32)
    wGr[:, :32] = wH * Gr[:KF, :32]
    wGi[:, :32] = -wH * Gi[:KF, :32]
    CT2a = np.zeros((128, 128), F32)   # T2 lhsT=Ure: rows (q,k2f), cols (q,c)
    CT2b = np.zeros((128, 128), F32)   # T2 lhsT=Uim
    for q in range(2):
        CT2a[KF * q:KF * q + KF, 64 * q:64 * q + 64] = wGr
        CT2b[KF * q:KF * q + KF, 64 * q:64 * q + 64] = wGi

    C16 = np.concatenate([CF2, CFh2a, CFh2b, CT1a, CT1b, CT2a, CT2b],
                         axis=1).astype(F16)

    Tq = np.zeros((128, 128), F32)     # loss weights (0.5*T2D per lane corner)
    th = _t2d_half()
    for b in range(2):
        for q in range(2):
            Tq[64 * b:64 * b + 32, 64 * q:64 * q + 32] = th
    Bind = np.zeros((128, 2), F32)     # partition-block indicator for colsums
    Bind[0:64, 0] = 1.0
    Bind[64:128, 1] = 1.0
    C32 = np.concatenate([Tq, Bind], axis=1).astype(F32)
    return C16, C32


# ---------------------------------------------------------------- bass program
def build_nc():
    import concourse.mybir as mybir
    import concourse.tile as tile
    from concourse import bacc

    f32 = mybir.dt.float32
    f16 = mybir.dt.float16
    Alu = mybir.AluOpType

    nc = bacc.Bacc("TRN2", target_bir_lowering=False)

    iq_d = nc.dram_tensor("iq", [128, 256], f16, kind="ExternalInput").ap()
    out_d = nc.dram_tensor("loss", [2, 2], f32, kind="ExternalOutput").ap()

    C16np, C32np = _consts()
    c16_d = nc.inline_tensor(C16np, "c16").ap()
    c32_d = nc.inline_tensor(C32np, "c32").ap()
    eps3 = float(EPS * SC ** 3)

    with tile.TileContext(nc) as tc:
        with (
            tc.tile_pool(name="consts", bufs=1) as consts,
            tc.tile_pool(name="state", bufs=1) as state,
            tc.tile_pool(name="loop", bufs=2) as loop,
            tc.tile_pool(name="psA", bufs=1, space="PSUM") as psA,
            tc.tile_pool(name="psB", bufs=2, space="PSUM") as psB,
            tc.tile_pool(name="psC", bufs=1, space="PSUM") as psC,
            tc.tile_pool(name="psD", bufs=2, space="PSUM") as psD,
            tc.tile_pool(name="psS", bufs=1, space="PSUM") as psS,
            tc.tile_pool(name="psJ", bufs=1, space="PSUM") as psJ,
        ):
            # ------------- tiles
            C16 = consts.tile([128, C16np.shape[1]], f16)
            C32 = consts.tile([128, C32np.shape[1]], f32)
            IQ = consts.tile([128, 256], f16)
            offs = np.cumsum([0, 256, 132, 132, 256, 256, 128, 128])
            CF2, CFh2a, CFh2b, CT1a, CT1b, CT2a, CT2b = (
                C16[:, int(offs[i]):int(offs[i + 1])] for i in range(7))
            Tq = C32[:, 0:128]
            Bind = C32[:, 128:130]

            junk = consts.tile([128, 256], f16)
            BindT = consts.tile([2, 128], f32)
            sqw = consts.tile([2, 4], f32)

            # warm-up + const DMAs first
            nc.any.memset(junk[:], 0.0)
            nc.any.memset(BindT[:], 0.0)
            nc.any.memset(BindT[:, 64:128], 1.0)
            nc.any.memset(BindT[0:1, 64:128], 0.0)
            nc.any.memset(BindT[0:1, 0:64], 1.0)
            nc.any.memset(sqw[:], 1.0)
            nc.scalar.sqrt(sqw[:], sqw[:])  # preload sqrt act table
            nc.sync.dma_start(C16[:], c16_d)
            nc.sync.dma_start(IQ[:], iq_d)
            nc.sync.dma_start(C32[:], c32_d)
            pj = psJ.tile([128, 256], f32, tag="pj")
            NWARM = 14
            for i in range(NWARM):  # HAM warm-up burst during DMA wait
                nc.tensor.matmul(pj[:], lhsT=junk[:, 0:128], rhs=junk[:],
                                 start=(i == 0), stop=(i == NWARM - 1))

            # persistent CG state
            rva = state.tile([128, 128], f16)
            rvb = state.tile([128, 128], f16)
            wva = state.tile([128, 128], f16)
            wvb = state.tile([128, 128], f16)
            zv = state.tile([128, 128], f32)
            sv = state.tile([128, 128], f32)
            pv = state.tile([128, 128], f32)
            xv = state.tile([128, 128], f32)
            Dall = state.tile([128, 66], f32)

            def qv(t):  # [128, (q,c)] -> [128, 2, 64]
                return t[:].rearrange("p (q c) -> p q c", q=2)

            def reim(ps):  # psum [128,(q,ri,33)] -> (re view, im view)
                v = ps[:].rearrange("p (q x k) -> p q x k", q=2, x=2)
                return v[:, :, 0, :], v[:, :, 1, :]

            def wq_views(t):  # Wq [128,(ri,(q,k))]: re cols 0:66, im 66:132
                return (t[:, 0:66].rearrange("p (q k) -> p q k", q=2),
                        t[:, 66:132].rearrange("p (q k) -> p q k", q=2))

            def fwd_fft(src_ap, tagp):
                """S1+S2 of a fp16 [128,128] quad -> psum [128,(q,ri,33)]."""
                ps1 = psA.tile([128, 256], f32, tag="psA")
                nc.tensor.matmul(ps1[:], lhsT=src_ap, rhs=CF2, start=True,
                                 stop=True)
                Hsb = loop.tile([128, 256], f16, tag=f"hsb{tagp}")
                nc.scalar.copy(Hsb[:, 0:128], ps1[:, 0:128])
                nc.vector.tensor_copy(Hsb[:, 128:256], ps1[:, 128:256])
                ps2 = psB.tile([128, 132], f32, tag="psB")
                for b in range(2):
                    dst = ps2[64 * b:64 * b + 64, :]
                    nc.tensor.matmul(dst, lhsT=Hsb[:, 128 * b:128 * b + 64],
                                     rhs=CFh2a, start=True, stop=False)
                    nc.tensor.matmul(dst,
                                     lhsT=Hsb[:, 128 * b + 64:128 * b + 128],
                                     rhs=CFh2b, start=False, stop=True)
                return ps2

            def inv_fft(Wq):
                """T1+T2 of fp16 Wq [128,132] -> psum [128,128] quad grid
                (exactly zero outside the 32x32 corners)."""
                ps3 = psC.tile([66, 256], f32, tag="psC")
                nc.tensor.matmul(ps3[:], lhsT=Wq[:, 0:66], rhs=CT1a,
                                 start=True, stop=False)
                nc.tensor.matmul(ps3[:], lhsT=Wq[:, 66:132], rhs=CT1b,
                                 start=False, stop=True)
                Tsb = loop.tile([66, 256], f16, tag="tsb")
                nc.scalar.copy(Tsb[:, 0:128], ps3[:, 0:128])
                nc.vector.tensor_copy(Tsb[:, 128:256], ps3[:, 128:256])
                ps4 = psD.tile([128, 128], f32, tag="psD")
                nc.tensor.matmul(ps4[:], lhsT=Tsb[:, 0:128],
                                 rhs=CT2a[0:66, :], start=True, stop=False)
                nc.tensor.matmul(ps4[:], lhsT=Tsb[:, 128:256],
                                 rhs=CT2b[0:66, :], start=False, stop=True)
                return ps4

            def matvec(src_grid):
                """raw BTTB matvec (no +eps): fp16 grid -> psum [128,128]."""
                ps2 = fwd_fft(src_grid[:], "m")
                psre, psim = reim(ps2)
                Wq = loop.tile([128, 132], f16, tag="wq")
                wre, wim = wq_views(Wq)
                dv = Dall[:].rearrange("p (q k) -> p q k", q=2)
                nc.vector.tensor_tensor(wre, psre, dv, op=Alu.mult)
                nc.vector.tensor_tensor(wim, psim, dv, op=Alu.mult)
                return inv_fft(Wq)

            # ------------- setup: FFT(X), FFT(d), D, b, w0 = A b
            ps2X = fwd_fft(IQ[:, 0:128], "x")
            Xsb = loop.tile([128, 132], f32, tag="xsb")
            nc.scalar.copy(Xsb[:, 0:66], ps2X[:, 0:66])
            nc.vector.tensor_copy(Xsb[:, 66:132], ps2X[:, 66:132])
            ps2R = fwd_fft(IQ[:, 128:256], "r")
            Xre, Xim = reim(Xsb)
            Rre, Rim = reim(ps2R)

            # bhat = conj(Xhat) * dhat -> Wq staging (fp16)
            Wqb = loop.tile([128, 132], f16, tag="wq")
            bre, bim = wq_views(Wqb)
            t1 = loop.tile([128, 66], f32, tag="t1")
            t2 = loop.tile([128, 66], f32, tag="t2")
            t3 = loop.tile([128, 66], f32, tag="t3")
            t4 = loop.tile([128, 66], f32, tag="t4")
            v1 = t1[:].rearrange("p (q k) -> p q k", q=2)
            v2 = t2[:].rearrange("p (q k) -> p q k", q=2)
            v3 = t3[:].rearrange("p (q k) -> p q k", q=2)
            v4 = t4[:].rearrange("p (q k) -> p q k", q=2)
            nc.vector.tensor_tensor(v1, Xre, Rre, op=Alu.mult)
            nc.vector.tensor_tensor(v2, Xim, Rim, op=Alu.mult)
            nc.vector.tensor_tensor(bre, v1, v2, op=Alu.add)
            nc.vector.tensor_tensor(v3, Xre, Rim, op=Alu.mult)
            nc.vector.tensor_tensor(v4, Xim, Rre, op=Alu.mult)
            nc.vector.tensor_tensor(bim, v3, v4, op=Alu.subtract)

            # D = |Xhat|^2 (scale SC^2 already inside Xhat)  [gpsimd]
            u1 = loop.tile([128, 66], f32, tag="u1")
            u2 = loop.tile([128, 66], f32, tag="u2")
            uv1 = u1[:].rearrange("p (q k) -> p q k", q=2)
            uv2 = u2[:].rearrange("p (q k) -> p q k", q=2)
            dv0 = Dall[:].rearrange("p (q k) -> p q k", q=2)
            nc.gpsimd.tensor_tensor(uv1, Xre, Xre, op=Alu.mult)
            nc.gpsimd.tensor_tensor(uv2, Xim, Xim, op=Alu.mult)
            nc.gpsimd.tensor_tensor(dv0, uv1, uv2, op=Alu.add)

            ps4b = inv_fft(Wqb)                      # b quad grid
            nc.vector.tensor_copy(rva[:], ps4b[:])   # r0 = b (fp16)
            ps4w = matvec(rva)                       # A r0
            nc.vector.scalar_tensor_tensor(          # w0 = A r0 + eps*r0
                out=wva[:], in0=rva[:], scalar=eps3, in1=ps4w[:],
                op0=Alu.mult, op1=Alu.add)

            # ------------- GV pipelined CG
            r_cur, r_nxt = rva, rvb
            w_cur, w_nxt = wva, wvb
            rgp_c = state.tile([2, 2], f32, name="rgp0")
            rap_c = state.tile([2, 2], f32, name="rap0")
            for it in range(ITERS):
                last = it == ITERS - 1
                ps4q = None if last else matvec(w_cur)

                # dots: gamma = <r,r> (gpsimd prod), delta = <w,r> (vector)
                red = loop.tile([128, 4], f32, tag="red")
                jg = loop.tile([128, 128], f32, tag="jg")
                jv = loop.tile([128, 128], f32, tag="jv")
                nc.gpsimd.tensor_tensor(jg[:], r_cur[:], r_cur[:],
                                        op=Alu.mult)
                nc.vector.tensor_tensor(jv[:], w_cur[:], r_cur[:],
                                        op=Alu.mult)
                nc.vector.tensor_reduce(
                    red[:, 0:2], qv(jg), mybir.AxisListType.X, Alu.add)
                nc.vector.tensor_reduce(
                    red[:, 2:4], qv(jv), mybir.AxisListType.X, Alu.add)
                pssm = psS.tile([128, 8], f32, tag="pssm")
                nc.tensor.matmul(pssm[0:2, 0:4], lhsT=Bind, rhs=red[:],
                                 start=True, stop=True)
                gd = loop.tile([2, 4], f32, tag="gd")
                nc.vector.tensor_copy(gd[:], pssm[0:2, 0:4])
                gam = gd[:, 0:2]
                dlt = gd[:, 2:4]

                cf = loop.tile([2, 4], f32, tag="cf")
                if it == 0:
                    nc.vector.memset(cf[:, 0:2], 0.0)  # beta = 0
                    rd = loop.tile([2, 2], f32, tag="s1")
                    nc.vector.reciprocal(rd[:], dlt)
                    nc.vector.tensor_tensor(cf[:, 2:4], gam, rd[:],
                                            op=Alu.mult)
                else:
                    nc.vector.tensor_tensor(cf[:, 0:2], gam, rgp_c[:],
                                            op=Alu.mult)  # beta
                    s2 = loop.tile([2, 2], f32, tag="s2")
                    s3 = loop.tile([2, 2], f32, tag="s3")
                    s4 = loop.tile([2, 2], f32, tag="s4")
                    s5 = loop.tile([2, 2], f32, tag="s5")
                    nc.vector.tensor_tensor(s2[:], cf[:, 0:2], gam,
                                            op=Alu.mult)
                    nc.vector.tensor_tensor(s3[:], s2[:], rap_c[:],
                                            op=Alu.mult)
                    nc.vector.tensor_tensor(s4[:], dlt, s3[:],
                                            op=Alu.subtract)
                    nc.vector.reciprocal(s5[:], s4[:])
                    nc.vector.tensor_tensor(cf[:, 2:4], gam, s5[:],
                                            op=Alu.mult)  # alpha
                if not last:
                    nc.vector.reciprocal(rgp_c[:], gam)      # for next iter
                    nc.vector.reciprocal(rap_c[:], cf[:, 2:4])

                nc.tensor.matmul(pssm[:, 4:8], lhsT=BindT[:], rhs=cf[:],
                                 start=True, stop=True)
                coefs = loop.tile([128, 4], f32, tag="coefs")
                nc.vector.tensor_copy(coefs[:], pssm[:, 4:8])
                bb = coefs[:, 0:2][:, :, None].broadcast_to([128, 2, 64])
                ab = coefs[:, 2:4][:, :, None].broadcast_to([128, 2, 64])

                if it == 0:
                    # z = q + eps*w ; s = w ; p = r
                    nc.vector.scalar_tensor_tensor(
                        out=zv[:], in0=w_cur[:], scalar=eps3, in1=ps4q[:],
                        op0=Alu.mult, op1=Alu.add)
                    nc.gpsimd.tensor_copy(sv[:], w_cur[:])
                    nc.gpsimd.tensor_copy(pv[:], r_cur[:])
                    # x = alpha * p
                    nc.gpsimd.tensor_tensor(qv(xv), qv(pv), ab, op=Alu.mult)
                    # r' = r - alpha*s (s=w), w' = w - alpha*z  [vector]
                    ta = loop.tile([128, 128], f32, tag="ta")
                    nc.vector.tensor_tensor(qv(ta), qv(sv), ab, op=Alu.mult)
                    nc.vector.tensor_tensor(r_nxt[:], r_cur[:], ta[:],
                                            op=Alu.subtract)
                    tb = loop.tile([128, 128], f32, tag="tb")
                    nc.vector.tensor_tensor(qv(tb), qv(zv), ab, op=Alu.mult)
                    nc.vector.tensor_tensor(w_nxt[:], w_cur[:], tb[:],
                                            op=Alu.subtract)
                elif not last:
                    # z' = beta*z + eps*w + q   [vector, critical]
                    tz = loop.tile([128, 128], f32, tag="tz")
                    nc.vector.tensor_tensor(qv(tz), qv(zv), bb, op=Alu.mult)
                    nc.vector.scalar_tensor_tensor(
                        out=tz[:], in0=w_cur[:], scalar=eps3, in1=tz[:],
                        op0=Alu.mult, op1=Alu.add)
                    nc.vector.tensor_tensor(zv[:], tz[:], ps4q[:], op=Alu.add)
                    # s' = w + beta*s ; p' = r + beta*p ; x += alpha*p' [gpsimd]
                    ts = loop.tile([128, 128], f32, tag="ts")
                    nc.gpsimd.tensor_tensor(qv(ts), qv(sv), bb, op=Alu.mult)
                    nc.gpsimd.tensor_tensor(sv[:], w_cur[:], ts[:], op=Alu.add)
                    tp = loop.tile([128, 128], f32, tag="tp")
                    nc.gpsimd.tensor_tensor(qv(tp), qv(pv), bb, op=Alu.mult)
                    nc.gpsimd.tensor_tensor(pv[:], r_cur[:], tp[:], op=Alu.add)
                    tx = loop.tile([128, 128], f32, tag="tx")
                    nc.gpsimd.tensor_tensor(qv(tx), qv(pv), ab, op=Alu.mult)
                    nc.gpsimd.tensor_tensor(xv[:], xv[:], tx[:], op=Alu.add)
                    # r' = r - alpha*s'  [gpsimd]
                    tr = loop.tile([128, 128], f32, tag="tr")
                    nc.gpsimd.tensor_tensor(qv(tr), qv(sv), ab, op=Alu.mult)
                    nc.gpsimd.tensor_tensor(r_nxt[:], r_cur[:], tr[:],
                                            op=Alu.subtract)
                    # w' = w - alpha*z'  [vector, critical]
                    tw = loop.tile([128, 128], f32, tag="tw")
                    nc.vector.tensor_tensor(qv(tw), qv(zv), ab, op=Alu.mult)
                    nc.vector.tensor_tensor(w_nxt[:], w_cur[:], tw[:],
                                            op=Alu.subtract)
                else:
                    # final: p' = r + beta*p ; x += alpha*p'
                    tp = loop.tile([128, 128], f32, tag="tp")
                    nc.gpsimd.tensor_tensor(qv(tp), qv(pv), bb, op=Alu.mult)
                    nc.gpsimd.tensor_tensor(pv[:], r_cur[:], tp[:], op=Alu.add)
                    tx = loop.tile([128, 128], f32, tag="tx")
                    nc.gpsimd.tensor_tensor(qv(tx), qv(pv), ab, op=Alu.mult)
                    nc.gpsimd.tensor_tensor(xv[:], xv[:], tx[:], op=Alu.add)

                r_cur, r_nxt = r_nxt, r_cur
                w_cur, w_nxt = w_nxt, w_cur

            # ------------- loss = sqrt(num)/sqrt(den) per lane
            twt = loop.tile([128, 128], f32, tag="twt")
            nc.vector.tensor_tensor(twt[:], xv[:], Tq, op=Alu.mult)
            red2 = loop.tile([128, 4], f32, tag="red2")
            jl1 = loop.tile([128, 128], f32, tag="jl1")
            jl2 = loop.tile([128, 128], f32, tag="jl2")
            nc.vector.tensor_tensor(jl1[:], twt[:], twt[:], op=Alu.mult)
            nc.gpsimd.tensor_tensor(jl2[:], xv[:], xv[:], op=Alu.mult)
            nc.vector.tensor_reduce(
                red2[:, 0:2], qv(jl1), mybir.AxisListType.X, Alu.add)
            nc.vector.tensor_reduce(
                red2[:, 2:4], qv(jl2), mybir.AxisListType.X, Alu.add)
            psl = psS.tile([128, 8], f32, tag="pssm")
            nc.tensor.matmul(psl[0:2, 0:4], lhsT=Bind, rhs=red2[:],
                             start=True, stop=True)
            ns = loop.tile([2, 4], f32, tag="ns")
            nc.vector.tensor_copy(ns[:], psl[0:2, 0:4])
            ns2 = loop.tile([2, 4], f32, tag="ns2")
            nc.scalar.sqrt(ns2[:], ns[:])
            rdn = loop.tile([2, 2], f32, tag="rdn")
            nc.vector.reciprocal(rdn[:], ns2[:, 2:4])
            loss_sb = loop.tile([2, 2], f32, tag="lsb")
            nc.vector.tensor_tensor(loss_sb[:], ns2[:, 0:2], rdn[:],
                                    op=Alu.mult)
            nc.sync.dma_start(out_d, loss_sb[:])

    return nc


def get_nc():
    if "nc" not in _NC_CACHE:
        nc = build_nc()
        if not nc.is_finalized():
            nc.finalize()
        _NC_CACHE["nc"] = nc
    return _NC_CACHE["nc"]


def pack_inputs(recon: np.ndarray, target: np.ndarray):
    """FULL inputs [8,3,32,32] -> per-core in_maps with quad-packed grids."""
    rec = np.asarray(recon, dtype=F32).reshape(24, H, W)
    tgt = np.asarray(target, dtype=F32).reshape(24, H, W)
    in_maps = []
    for c in range(N_CORES):
        lanes = [3 * c, 3 * c + 1, 3 * c + 2, 3 * c + 2]
        IQ = np.zeros((128, 256), F16)
        for j in range(4):
            b, q = j >> 1, j & 1
            IQ[64 * b:64 * b + 32, 64 * q:64 * q + 32] = tgt[lanes[j]]
            IQ[64 * b + 15:64 * b + 47,
               128 + 64 * q + 15:128 + 64 * q + 47] = rec[lanes[j]]
        in_maps.append({"iq": IQ})
    return in_maps


# ---------------------------------------------------------------- entry point
def kernel(recon: np.ndarray, target: np.ndarray) -> np.ndarray:
    from concourse.bass_utils import run_bass_kernel_spmd

    in_maps = pack_inputs(recon, target)
    nc = get_nc()
    res = run_bass_kernel_spmd(nc, in_maps, list(range(N_CORES)))
    total = F32(0.0)
    for c in range(N_CORES):
        L = res.results[c]["loss"].astype(F32)
        total += L[0, 0] + L[0, 1] + L[1, 0]
    return np.asarray(total, dtype=F32)
